# revision 1
# baseline (speedup 1.0000x reference)
"""Trainium2 Bass kernel for 10-layer LSTM + additive attention pooling + FC.

Sharding: data-parallel over batch (8 cores x 32). Per core all 10 layers run
as a wavefront (layer l computes step t = tick - l). Gates are batch-major
[32b x 512] per layer; 4 layers share one PSUM bank stacked on partitions.
Per (layer,step): x-part matmul (stationary h_{l-1}(t)), recurrent matmul
(stationary h_l(t-1)), K=1 bias matmul. Nonlinearities/cell updates are
merged across all 10 layers into single strided ACT/DVE instructions.
h returns to H-major via one PE transpose per 4-layer group per tick.
"""
import sys
import numpy as np

B, S, IN, H, OUT, L = 256, 512, 27, 128, 7, 10
NCORES = 8
BC = B // NCORES  # 32
G4 = 4 * H        # 512

for _p in ("/opt/trn_rl_repo",):
    if _p not in sys.path:
        sys.path.insert(0, _p)

_CACHE = {}


def _build(S_run):
    from contextlib import ExitStack
    import concourse.bass as bass
    import concourse.tile as tile
    from concourse import bacc, mybir
    from concourse.masks import make_identity

    f32 = mybir.dt.float32
    bf16 = mybir.dt.bfloat16
    fp16 = mybir.dt.float16
    NT = S_run + L - 1

    nc = bacc.Bacc("TRN2", target_bir_lowering=False, debug=False,
                   enable_asserts=False, num_devices=NCORES)

    d_x = nc.dram_tensor("x", [IN + 1, S_run * BC], fp16, kind="ExternalInput").ap()
    d_w0 = nc.dram_tensor("w0", [IN + 1, G4], fp16, kind="ExternalInput").ap()
    d_wx = nc.dram_tensor("wx", [128, 9 * G4], f32, kind="ExternalInput").ap()
    d_wh = nc.dram_tensor("wh", [128, 10 * G4], f32, kind="ExternalInput").ap()
    d_bias = nc.dram_tensor("bias", [1, 9 * G4], f32, kind="ExternalInput").ap()
    d_attn = nc.dram_tensor("attn_wT", [128, 128], f32, kind="ExternalInput").ap()
    d_attnb = nc.dram_tensor("attn_b", [128, 1], f32, kind="ExternalInput").ap()
    d_vw = nc.dram_tensor("v_w", [128, 1], f32, kind="ExternalInput").ap()
    d_fcw = nc.dram_tensor("fc_wT", [128, OUT], f32, kind="ExternalInput").ap()
    d_fcb = nc.dram_tensor("fc_b", [1, OUT], f32, kind="ExternalInput").ap()
    d_out = nc.dram_tensor("out", [OUT, BC], f32, kind="ExternalOutput").ap()
    import os as _os
    dbg = _os.environ.get("DEBUG_HS9") == "1"
    d_hs9 = (nc.dram_tensor("hs9", [128, S_run * BC], f32,
                            kind="ExternalOutput").ap() if dbg else None)

    Sig = mybir.ActivationFunctionType.Sigmoid
    Tanh = mybir.ActivationFunctionType.Tanh
    Exp = mybir.ActivationFunctionType.Exp

    with tile.TileContext(nc) as tc:
        with ExitStack() as octx:
            keep = octx.enter_context(tc.tile_pool(name="keep", bufs=1))
            hs9 = keep.tile([128, S_run * BC], f32)
            ident = keep.tile([128, 128], f32)
            make_identity(nc, ident[:])

            # ================= recurrent phase =================
            with ExitStack() as ctx:
                stat = ctx.enter_context(tc.tile_pool(name="stat", bufs=1))
                xT = stat.tile([IN + 1, S_run * BC], fp16)
                nc.sync.dma_start(xT[:], d_x)
                w0 = stat.tile([IN + 1, G4], fp16)
                nc.sync.dma_start(w0[:], d_w0)
                Wx = stat.tile([128, 9 * G4], f32)
                nc.sync.dma_start(Wx[:], d_wx)
                Wh = stat.tile([128, 10 * G4], f32)
                nc.sync.dma_start(Wh[:], d_wh)
                bias_sb = stat.tile([1, 9 * G4], f32)
                nc.sync.dma_start(bias_sb[:], d_bias)
                ones32 = stat.tile([1, 32], f32)
                nc.vector.memset(ones32[:], 1.0)

                psum = ctx.enter_context(tc.tile_pool(name="ps", bufs=2,
                                                      space="PSUM"))
                pst = ctx.enter_context(tc.tile_pool(name="pst", bufs=2,
                                                     space="PSUM"))
                actp = ctx.enter_context(tc.tile_pool(name="act", bufs=2))
                hbp = ctx.enter_context(tc.tile_pool(name="hb", bufs=2))
                htp = ctx.enter_context(tc.tile_pool(name="ht", bufs=3))
                tmpp = ctx.enter_context(tc.tile_pool(name="tmp", bufs=2))
                thp = ctx.enter_context(tc.tile_pool(name="th", bufs=2))
                cpp = ctx.enter_context(tc.tile_pool(name="cp", bufs=1))

                c_t = cpp.tile([128, 384], f32)
                hT_prev = None

                def cell(ps_ifo, ps_g, a_ifo, a_g, i_, f_, o_, g_,
                         cc, tt, tth, hh, t0):
                    nc.scalar.activation(a_ifo, ps_ifo, Sig)
                    nc.scalar.activation(a_g, ps_g, Tanh)
                    nc.vector.tensor_mul(tt, i_, g_)
                    if t0:
                        nc.vector.tensor_copy(cc, tt)
                    else:
                        nc.vector.tensor_mul(cc, f_, cc)
                        nc.vector.tensor_add(cc, cc, tt)
                    nc.scalar.activation(tth, cc, Tanh)
                    nc.vector.tensor_mul(hh, o_, tth)

                for k in range(NT):
                    act_l = [l for l in range(L) if 0 <= k - l < S_run]
                    full = len(act_l) == L
                    ps = psum.tile([128, 3 * G4], f32)
                    for l in act_l:
                        t = k - l
                        g, m = l // 4, l % 4
                        pr = slice(32 * m, 32 * m + 32)
                        o = ps[pr, G4 * g:G4 * (g + 1)]
                        tp = (0, 32 * m)
                        if l == 0:
                            lx, wx_r = xT[:, 32 * t:32 * t + 32], w0[:]
                        else:
                            lx = hT_prev[:, 32 * (l - 1):32 * (l - 1) + 32]
                            wx_r = Wx[:, (l - 1) * G4:l * G4]
                        nc.tensor.matmul(o, lx, wx_r, start=True,
                                         stop=(t == 0 and l == 0),
                                         tile_position=tp)
                        if t > 0:
                            nc.tensor.matmul(
                                o, hT_prev[:, 32 * l:32 * l + 32],
                                Wh[:, l * G4:(l + 1) * G4],
                                start=False, stop=(l == 0), tile_position=tp)
                        if l > 0:
                            nc.tensor.matmul(
                                o, ones32[:], bias_sb[:, (l - 1) * G4:l * G4],
                                start=False, stop=True, tile_position=tp)

                    act = actp.tile([128, 3 * G4], f32)
                    tmp = tmpp.tile([128, 384], f32)
                    th = thp.tile([128, 384], f32)
                    h_b = hbp.tile([128, 384], f32)
                    if full:
                        p3 = ps[:].rearrange("p (g c) -> p g c", g=3)
                        a3 = act[:].rearrange("p (g c) -> p g c", g=3)
                        c3 = c_t[:].rearrange("p (g c) -> p g c", g=3)
                        t3 = tmp[:].rearrange("p (g c) -> p g c", g=3)
                        h3 = th[:].rearrange("p (g c) -> p g c", g=3)
                        b3 = h_b[:].rearrange("p (g c) -> p g c", g=3)
                        cell(p3[:, :, 0:384], p3[:, :, 384:512],
                             a3[:, :, 0:384], a3[:, :, 384:512],
                             a3[:, :, 0:128], a3[:, :, 128:256],
                             a3[:, :, 256:384], a3[:, :, 384:512],
                             c3[:, :, :], t3[:, :, :], h3[:, :, :],
                             b3[:, :, :], False)
                    else:
                        for l in act_l:
                            t = k - l
                            g, m = l // 4, l % 4
                            pr = slice(32 * m, 32 * m + 32)
                            pc = ps[pr, G4 * g:G4 * (g + 1)]
                            ac = act[pr, G4 * g:G4 * (g + 1)]
                            cc = c_t[pr, 128 * g:128 * (g + 1)]
                            tc_ = tmp[pr, 128 * g:128 * (g + 1)]
                            hc = th[pr, 128 * g:128 * (g + 1)]
                            bc = h_b[pr, 128 * g:128 * (g + 1)]
                            cell(pc[:, 0:384], pc[:, 384:512],
                                 ac[:, 0:384], ac[:, 384:512],
                                 ac[:, 0:128], ac[:, 128:256],
                                 ac[:, 256:384], ac[:, 384:512],
                                 cc, tc_, hc, bc, t == 0)

                    hT = htp.tile([128, 384], f32)
                    for g in sorted(set(l // 4 for l in act_l)):
                        pt = pst.tile([128, 128], f32)
                        nc.tensor.transpose(
                            pt[:], h_b[:, 128 * g:128 * (g + 1)], ident[:])
                        nc.vector.tensor_copy(
                            hT[:, 128 * g:128 * (g + 1)], pt[:])
                    if 9 in act_l:
                        t9 = k - 9
                        nc.gpsimd.tensor_copy(
                            hs9[:, 32 * t9:32 * t9 + 32], hT[:, 288:320])
                    hT_prev = hT

            if d_hs9 is not None:
                nc.sync.dma_start(d_hs9, hs9[:])
            # ================= attention + FC =================
            with ExitStack() as ctx:
                st2 = ctx.enter_context(tc.tile_pool(name="st2", bufs=1))
                ps2 = ctx.enter_context(tc.tile_pool(name="ps2", bufs=2,
                                                     space="PSUM"))
                sc2 = ctx.enter_context(tc.tile_pool(name="sc2", bufs=2))
                aw = st2.tile([128, 128], f32)
                nc.sync.dma_start(aw[:], d_attn)
                ab = st2.tile([128, 1], f32)
                nc.sync.dma_start(ab[:], d_attnb)
                vw = st2.tile([128, 1], f32)
                nc.sync.dma_start(vw[:], d_vw)
                fcw = st2.tile([128, OUT], f32)
                nc.sync.dma_start(fcw[:], d_fcw)
                fcb = st2.tile([1, OUT], f32)
                nc.sync.dma_start(fcb[:], d_fcb)
                ones128 = st2.tile([1, 128], f32)
                nc.vector.memset(ones128[:], 1.0)
                onesBC = st2.tile([1, BC], f32)
                nc.vector.memset(onesBC[:], 1.0)

                NCH = (S_run * BC) // 512
                wgt = st2.tile([1, S_run * BC], f32)
                for ch in range(NCH):
                    cs = slice(512 * ch, 512 * (ch + 1))
                    pa = ps2.tile([128, 512], f32, tag="big")
                    nc.tensor.matmul(pa[:], aw[:], hs9[:, cs],
                                     start=True, stop=True)
                    sc = sc2.tile([128, 512], f32)
                    nc.scalar.activation(sc[:], pa[:], Tanh, bias=ab[:])
                    pl = ps2.tile([1, 512], f32, tag="pl")
                    nc.tensor.matmul(pl[:], vw[:], sc[:],
                                     start=True, stop=True)
                    nc.scalar.activation(wgt[:, cs], pl[:], Exp)
                # unnormalized weighted sum + per-b normalization at the end
                sm = st2.tile([1, BC], f32)
                nc.vector.tensor_reduce(
                    sm[:], wgt[:].rearrange("p (t b) -> p b t", b=BC),
                    axis=mybir.AxisListType.X, op=mybir.AluOpType.add)
                rsm = st2.tile([1, BC], f32)
                nc.vector.reciprocal(rsm[:], sm[:])

                parts = st2.tile([128, NCH * BC], f32)
                for ch in range(NCH):
                    cs = slice(512 * ch, 512 * (ch + 1))
                    pw = ps2.tile([128, 512], f32, tag="big")
                    nc.tensor.matmul(pw[:], ones128[:], wgt[:, cs],
                                     start=True, stop=True)
                    wp = sc2.tile([128, 512], f32, tag="wp")
                    nc.vector.tensor_mul(wp[:], hs9[:, cs], pw[:])
                    nc.vector.tensor_reduce(
                        parts[:, BC * ch:BC * (ch + 1)],
                        wp[:].rearrange("p (t b) -> p b t", b=BC),
                        axis=mybir.AxisListType.X, op=mybir.AluOpType.add)
                ctxv = st2.tile([128, BC], f32)
                nc.vector.tensor_reduce(
                    ctxv[:], parts[:].rearrange("p (c b) -> p b c", b=BC),
                    axis=mybir.AxisListType.X, op=mybir.AluOpType.add)
                prn = ps2.tile([128, BC], f32, tag="pl")
                nc.tensor.matmul(prn[:], ones128[:], rsm[:],
                                 start=True, stop=True)
                nc.vector.tensor_mul(ctxv[:], ctxv[:], prn[:])

                pf = ps2.tile([OUT, BC], f32, tag="pl")
                nc.tensor.matmul(pf[:], fcw[:], ctxv[:],
                                 start=True, stop=False)
                nc.tensor.matmul(pf[:], fcb[:], onesBC[:],
                                 start=False, stop=True)
                ov = sc2.tile([OUT, BC], f32, tag="ov")
                nc.vector.tensor_copy(ov[:], pf[:])
                nc.sync.dma_start(d_out, ov[:])

    nc.compile()
    return nc


def _prep_inputs(x, w_ih0, w_ih, w_hh, b_ih, b_hh, attn_w, attn_b, v_w, v_b,
                 fc_w, fc_b, S_run):
    bf = np.float16
    perm = np.concatenate([np.arange(0, H), np.arange(H, 2 * H),
                           np.arange(3 * H, 4 * H), np.arange(2 * H, 3 * H)])
    w0 = np.concatenate([w_ih0.T, (b_ih[0] + b_hh[0])[None, :]], 0)[:, perm]
    wx = np.concatenate([w_ih[l - 1].T[:, perm] for l in range(1, L)], 1)
    wh = np.concatenate([w_hh[l].T[:, perm] for l in range(L)], 1)
    bias = np.concatenate([(b_ih[l] + b_hh[l])[perm][None, :]
                           for l in range(1, L)], 1)
    shared = {
        "w0": np.ascontiguousarray(w0).astype(bf),
        "wx": np.ascontiguousarray(wx, np.float32),
        "wh": np.ascontiguousarray(wh, np.float32),
        "bias": np.ascontiguousarray(bias, np.float32),
        "attn_wT": np.ascontiguousarray(attn_w.T, np.float32),
        "attn_b": np.ascontiguousarray(attn_b[:, None], np.float32),
        "v_w": np.ascontiguousarray(v_w.T, np.float32),
        "fc_wT": np.ascontiguousarray(fc_w.T, np.float32),
        "fc_b": np.ascontiguousarray(fc_b[None, :], np.float32),
    }
    in_maps = []
    for c in range(NCORES):
        xs = x[c * BC:(c + 1) * BC, :S_run, :]
        xt = np.transpose(xs, (2, 1, 0)).reshape(IN, S_run * BC)
        xt = np.concatenate([xt, np.ones((1, S_run * BC), np.float32)], 0)
        m = dict(shared)
        m["x"] = np.ascontiguousarray(xt).astype(bf)
        in_maps.append(m)
    return in_maps


def run(inputs, S_run=S, trace=False):
    from concourse import bass_utils
    if S_run not in _CACHE:
        _CACHE[S_run] = _build(S_run)
    nc = _CACHE[S_run]
    in_maps = _prep_inputs(S_run=S_run, **inputs)
    res = bass_utils.run_bass_kernel_spmd(
        nc, in_maps, core_ids=list(range(NCORES)), trace=trace)
    out = np.concatenate([np.asarray(res.results[c]["out"], np.float32).T
                          for c in range(NCORES)], 0)
    return np.ascontiguousarray(out, np.float32), res


def kernel(**inputs):
    inputs = {k: np.asarray(v, np.float32) for k, v in inputs.items()}
    out, _ = run(inputs, S_run=S)
    return out



# revision 9
# speedup vs baseline: 4.9095x; 4.9095x over previous
"""Trainium2 Bass kernel for 10-layer LSTM + additive attention pooling + FC.

Sharding: data-parallel over batch (8 cores x 32). Per core all 10 layers run
as a wavefront (layer l computes step t = tick - l). Gates are batch-major
[4 layers x 32b, 512 gate] per PSUM bank; layers are grouped by bank
(0-3 / 4-7 / 8-9) and the three groups are software-pipelined so ACT/DVE
cell updates of one group overlap PE matmuls of the next. All matmul
operands are fp16 (1 cycle/row streaming vs 4 for fp32); PSUM accumulation
stays fp32 and the cell state c stays fp32 in SBUF. Per-layer K=1 bias
matmuls are replaced by one "expander" matmul per bank: stationary E[4,128]
with E[j, 32j:32j+32]=1 broadcasts 4 layers' bias rows across the bank's
partitions in a single 512-column pass.
"""
import sys
import numpy as np

B, S, IN, H, OUT, L = 256, 512, 27, 128, 7, 10
NCORES = 8
BC = B // NCORES  # 32
G4 = 4 * H        # 512

for _p in ("/opt/trn_rl_repo",):
    if _p not in sys.path:
        sys.path.insert(0, _p)

_CACHE = {}

GROUPS = ((0, 1, 2, 3), (4, 5, 6, 7), (8, 9))


def _build(S_run):
    from contextlib import ExitStack
    import concourse.bass as bass
    import concourse.tile as tile
    from concourse import bacc, mybir
    from concourse.masks import make_identity

    f32 = mybir.dt.float32
    fp16 = mybir.dt.float16
    NT = S_run + L - 1

    nc = bacc.Bacc("TRN2", target_bir_lowering=False, debug=False,
                   enable_asserts=False, num_devices=NCORES)

    d_x = nc.dram_tensor("x", [IN + 1, S_run * BC], fp16, kind="ExternalInput").ap()
    d_w0 = nc.dram_tensor("w0", [IN + 1, G4], fp16, kind="ExternalInput").ap()
    d_wx = nc.dram_tensor("wx", [128, 9 * G4], fp16, kind="ExternalInput").ap()
    d_wh = nc.dram_tensor("wh", [128, 10 * G4], fp16, kind="ExternalInput").ap()
    d_brow = nc.dram_tensor("brow", [1, 9 * G4], fp16, kind="ExternalInput").ap()
    d_bbank = nc.dram_tensor("bbank", [4, 3 * G4], fp16, kind="ExternalInput").ap()
    d_E = nc.dram_tensor("Emat", [4, 128], fp16, kind="ExternalInput").ap()
    d_attn = nc.dram_tensor("attn_wT", [128, 128], fp16, kind="ExternalInput").ap()
    d_attnb = nc.dram_tensor("attn_b", [128, 1], f32, kind="ExternalInput").ap()
    d_vw = nc.dram_tensor("v_w", [128, 1], fp16, kind="ExternalInput").ap()
    d_fcw = nc.dram_tensor("fc_wT", [128, OUT], fp16, kind="ExternalInput").ap()
    d_fcb = nc.dram_tensor("fc_b", [1, OUT], fp16, kind="ExternalInput").ap()
    d_out = nc.dram_tensor("out", [OUT, BC], f32, kind="ExternalOutput").ap()
    import os as _os
    dbg = _os.environ.get("DEBUG_HS9") == "1"
    d_hs9 = (nc.dram_tensor("hs9", [128, S_run * BC], fp16,
                            kind="ExternalOutput").ap() if dbg else None)

    Sig = mybir.ActivationFunctionType.Sigmoid
    Tanh = mybir.ActivationFunctionType.Tanh
    Exp = mybir.ActivationFunctionType.Exp

    def active(k):
        return [l for l in range(L) if 0 <= k - l < S_run]

    with tile.TileContext(nc) as tc:
        with ExitStack() as octx:
            keep = octx.enter_context(tc.tile_pool(name="keep", bufs=1))
            hs9 = keep.tile([128, S_run * BC], fp16)
            ident = keep.tile([128, 128], fp16)
            make_identity(nc, ident[:])

            # ================= recurrent phase =================
            with ExitStack() as ctx:
                stat = ctx.enter_context(tc.tile_pool(name="stat", bufs=1))
                xT = stat.tile([IN + 1, S_run * BC], fp16)
                nc.sync.dma_start(xT[:], d_x)
                w0 = stat.tile([IN + 1, G4], fp16)
                nc.sync.dma_start(w0[:], d_w0)
                Wx = stat.tile([128, 9 * G4], fp16)
                nc.sync.dma_start(Wx[:], d_wx)
                Wh = stat.tile([128, 10 * G4], fp16)
                nc.sync.dma_start(Wh[:], d_wh)
                brow = stat.tile([1, 9 * G4], fp16)
                nc.sync.dma_start(brow[:], d_brow)
                bbank = stat.tile([4, 3 * G4], fp16)
                nc.sync.dma_start(bbank[:], d_bbank)
                ones32 = stat.tile([1, 32], fp16)
                nc.vector.memset(ones32[:], 1.0)
                E = stat.tile([4, 128], fp16)
                nc.sync.dma_start(E[:], d_E)

                psp = [ctx.enter_context(
                    tc.tile_pool(name=f"ps{g}", bufs=2, space="PSUM"))
                    for g in range(3)]
                pst = ctx.enter_context(tc.tile_pool(name="pst", bufs=2,
                                                     space="PSUM"))
                actp = [ctx.enter_context(tc.tile_pool(name=f"act{g}", bufs=2))
                        for g in range(3)]
                hbp = [ctx.enter_context(tc.tile_pool(name=f"hb{g}", bufs=2))
                       for g in range(3)]
                htp = [ctx.enter_context(tc.tile_pool(name=f"ht{g}", bufs=2))
                       for g in range(3)]
                tmpp = [ctx.enter_context(tc.tile_pool(name=f"tp{g}", bufs=2))
                        for g in range(3)]
                thp = [ctx.enter_context(tc.tile_pool(name=f"th{g}", bufs=2))
                       for g in range(3)]
                cpp = ctx.enter_context(tc.tile_pool(name="cp", bufs=1))

                c_g = [cpp.tile([128, 128], f32, name=f"c{g}")
                       for g in range(3)]
                for g in range(3):
                    nc.vector.memset(c_g[g][:], 0.0)

                # rolling state: hb tile of tick k-1, hT tile of tick k-1
                hb_prev = [None, None, None]
                hT_prev = [None, None, None]
                ps_cur = [None, None, None]

                def emit_C(g, k):
                    """transpose+copy h of tick k for group g -> hT_prev[g]."""
                    if k < 0 or hb_prev[g] is None:
                        return
                    pt = pst.tile([128, 128], fp16)
                    nc.tensor.transpose(pt[:], hb_prev[g][:], ident[:])
                    hT = htp[g].tile([128, 128], fp16)
                    nc.vector.tensor_copy(hT[:], pt[:])
                    hT_prev[g] = hT
                    if g == 2 and (0 <= k - 9 < S_run):
                        t9 = k - 9
                        nc.gpsimd.tensor_copy(
                            hs9[:, 32 * t9:32 * t9 + 32], hT[:, 32:64])

                def x_operands(l, t):
                    if l == 0:
                        return xT[:, 32 * t:32 * t + 32], w0[:]
                    gp, mp = (l - 1) // 4, (l - 1) % 4
                    return (hT_prev[gp][:, 32 * mp:32 * mp + 32],
                            Wx[:, (l - 1) * G4:l * G4])

                def emit_A(g, k, full):
                    lays = [l for l in GROUPS[g] if 0 <= k - l < S_run]
                    if not lays:
                        ps_cur[g] = None
                        return
                    ps = psp[g].tile([128, G4], f32)
                    ps_cur[g] = ps
                    if full:
                        for l in lays:
                            t, m = k - l, l % 4
                            tp = (0, 32 * m)
                            o = ps[32 * m:32 * m + 32, :]
                            lx, wx_r = x_operands(l, t)
                            nc.tensor.matmul(o, lx, wx_r, start=True,
                                             stop=False, tile_position=tp)
                            if t > 0:
                                nc.tensor.matmul(
                                    o, hT_prev[g][:, 32 * m:32 * m + 32],
                                    Wh[:, l * G4:(l + 1) * G4],
                                    start=False, stop=False, tile_position=tp)
                        nc.tensor.matmul(ps[:], E[:],
                                         bbank[:, g * G4:(g + 1) * G4],
                                         start=False, stop=True,
                                         skip_group_check=True)
                    else:
                        for l in lays:
                            t, m = k - l, l % 4
                            tp = (0, 32 * m)
                            o = ps[32 * m:32 * m + 32, :]
                            lx, wx_r = x_operands(l, t)
                            nc.tensor.matmul(
                                o, lx, wx_r, start=True,
                                stop=(l == 0 and t == 0), tile_position=tp)
                            if t > 0:
                                nc.tensor.matmul(
                                    o, hT_prev[g][:, 32 * m:32 * m + 32],
                                    Wh[:, l * G4:(l + 1) * G4],
                                    start=False, stop=(l == 0),
                                    tile_position=tp)
                            if l > 0:
                                nc.tensor.matmul(
                                    o, ones32[:], brow[:, (l - 1) * G4:l * G4],
                                    start=False, stop=True, tile_position=tp)

                def cell(pr, g):
                    """gate nonlinearities + cell update on partition range pr."""
                    ps, act = ps_cur[g], act_cur[g]
                    nc.scalar.activation(act[pr, 0:384], ps[pr, 0:384], Sig)
                    nc.scalar.activation(act[pr, 384:512], ps[pr, 384:512],
                                         Tanh)
                    tt, th, cc = tt_cur[g], th_cur[g], c_g[g]
                    nc.vector.tensor_mul(tt[pr, :], act[pr, 0:128],
                                         act[pr, 384:512])
                    nc.vector.tensor_mul(cc[pr, :], act[pr, 128:256],
                                         cc[pr, :])
                    nc.vector.tensor_add(cc[pr, :], cc[pr, :], tt[pr, :])
                    nc.scalar.activation(th[pr, :], cc[pr, :], Tanh)
                    nc.vector.tensor_mul(hb_prev[g][pr, :], act[pr, 256:384],
                                         th[pr, :])

                act_cur = [None, None, None]
                tt_cur = [None, None, None]
                th_cur = [None, None, None]

                def emit_B(g, k, full):
                    lays = [l for l in GROUPS[g] if 0 <= k - l < S_run]
                    if not lays:
                        hb_prev[g] = None
                        return
                    act_cur[g] = actp[g].tile([128, G4], f32, name=f"act{g}")
                    tt_cur[g] = tmpp[g].tile([128, 128], f32, name=f"tt{g}")
                    th_cur[g] = thp[g].tile([128, 128], f32, name=f"th{g}")
                    hb_prev[g] = hbp[g].tile([128, 128], fp16, name=f"hb{g}")
                    if full:
                        cell(slice(0, 32 * len(lays)), g)
                    else:
                        for l in lays:
                            m = l % 4
                            cell(slice(32 * m, 32 * m + 32), g)

                for k in range(NT):
                    for g in range(3):
                        lays = [l for l in GROUPS[g] if 0 <= k - l < S_run]
                        full = len(lays) == len(GROUPS[g])
                        emit_C(g, k - 1)
                        emit_A(g, k, full)
                        emit_B(g, k, full)
                # flush: hs9 for the final tick (layer 9, t = S_run-1)
                emit_C(2, NT - 1)

            if d_hs9 is not None:
                nc.sync.dma_start(d_hs9, hs9[:])
            # ================= attention + FC =================
            with ExitStack() as ctx:
                st2 = ctx.enter_context(tc.tile_pool(name="st2", bufs=1))
                ps2 = ctx.enter_context(tc.tile_pool(name="ps2", bufs=2,
                                                     space="PSUM"))
                sc2 = ctx.enter_context(tc.tile_pool(name="sc2", bufs=2))
                aw = st2.tile([128, 128], fp16)
                nc.sync.dma_start(aw[:], d_attn)
                ab = st2.tile([128, 1], f32)
                nc.sync.dma_start(ab[:], d_attnb)
                vw = st2.tile([128, 1], fp16)
                nc.sync.dma_start(vw[:], d_vw)
                fcw = st2.tile([128, OUT], fp16)
                nc.sync.dma_start(fcw[:], d_fcw)
                fcb = st2.tile([1, OUT], fp16)
                nc.sync.dma_start(fcb[:], d_fcb)
                ones128 = st2.tile([1, 128], fp16)
                nc.vector.memset(ones128[:], 1.0)
                onesBC = st2.tile([1, BC], fp16)
                nc.vector.memset(onesBC[:], 1.0)

                NCH = (S_run * BC) // 512
                wgt = st2.tile([1, S_run * BC], fp16)
                for ch in range(NCH):
                    cs = slice(512 * ch, 512 * (ch + 1))
                    pa = ps2.tile([128, 512], f32, tag="big")
                    nc.tensor.matmul(pa[:], aw[:], hs9[:, cs],
                                     start=True, stop=True)
                    sc = sc2.tile([128, 512], fp16)
                    nc.scalar.activation(sc[:], pa[:], Tanh, bias=ab[:])
                    pl = ps2.tile([1, 512], f32, tag="pl")
                    nc.tensor.matmul(pl[:], vw[:], sc[:],
                                     start=True, stop=True)
                    nc.scalar.activation(wgt[:, cs], pl[:], Exp)
                # unnormalized weighted sum + per-b normalization at the end
                sm = st2.tile([1, BC], f32)
                nc.vector.tensor_reduce(
                    sm[:], wgt[:].rearrange("p (t b) -> p b t", b=BC),
                    axis=mybir.AxisListType.X, op=mybir.AluOpType.add)
                rsm = st2.tile([1, BC], f32)
                nc.vector.reciprocal(rsm[:], sm[:])
                rsm16 = st2.tile([1, BC], fp16)
                nc.vector.tensor_copy(rsm16[:], rsm[:])

                parts = st2.tile([128, NCH * BC], f32)
                for ch in range(NCH):
                    cs = slice(512 * ch, 512 * (ch + 1))
                    pw = ps2.tile([128, 512], f32, tag="big")
                    nc.tensor.matmul(pw[:], ones128[:], wgt[:, cs],
                                     start=True, stop=True)
                    wp = sc2.tile([128, 512], f32, tag="wp")
                    nc.vector.tensor_mul(wp[:], hs9[:, cs], pw[:])
                    nc.vector.tensor_reduce(
                        parts[:, BC * ch:BC * (ch + 1)],
                        wp[:].rearrange("p (t b) -> p b t", b=BC),
                        axis=mybir.AxisListType.X, op=mybir.AluOpType.add)
                ctxv = st2.tile([128, BC], f32)
                nc.vector.tensor_reduce(
                    ctxv[:], parts[:].rearrange("p (c b) -> p b c", b=BC),
                    axis=mybir.AxisListType.X, op=mybir.AluOpType.add)
                prn = ps2.tile([128, BC], f32, tag="pl")
                nc.tensor.matmul(prn[:], ones128[:], rsm16[:],
                                 start=True, stop=True)
                nc.vector.tensor_mul(ctxv[:], ctxv[:], prn[:])
                ctx16 = st2.tile([128, BC], fp16)
                nc.vector.tensor_copy(ctx16[:], ctxv[:])

                pf = ps2.tile([OUT, BC], f32, tag="pl")
                nc.tensor.matmul(pf[:], fcw[:], ctx16[:],
                                 start=True, stop=False)
                nc.tensor.matmul(pf[:], fcb[:], onesBC[:],
                                 start=False, stop=True)
                ov = sc2.tile([OUT, BC], f32, tag="ov")
                nc.vector.tensor_copy(ov[:], pf[:])
                nc.sync.dma_start(d_out, ov[:])

    nc.compile()
    return nc


def _prep_inputs(x, w_ih0, w_ih, w_hh, b_ih, b_hh, attn_w, attn_b, v_w, v_b,
                 fc_w, fc_b, S_run):
    hf = np.float16
    perm = np.concatenate([np.arange(0, H), np.arange(H, 2 * H),
                           np.arange(3 * H, 4 * H), np.arange(2 * H, 3 * H)])
    w0 = np.concatenate([w_ih0.T, (b_ih[0] + b_hh[0])[None, :]], 0)[:, perm]
    wx = np.concatenate([w_ih[l - 1].T[:, perm] for l in range(1, L)], 1)
    wh = np.concatenate([w_hh[l].T[:, perm] for l in range(L)], 1)
    brow = np.concatenate([(b_ih[l] + b_hh[l])[perm][None, :]
                           for l in range(1, L)], 1)
    bbank = np.zeros((4, 3 * G4), np.float32)
    for l in range(1, L):
        g, j = l // 4, l % 4
        bbank[j, g * G4:(g + 1) * G4] = (b_ih[l] + b_hh[l])[perm]
    Emat = np.zeros((4, 128), np.float32)
    for j in range(4):
        Emat[j, 32 * j:32 * j + 32] = 1.0
    shared = {
        "Emat": Emat.astype(hf),
        "w0": np.ascontiguousarray(w0).astype(hf),
        "wx": np.ascontiguousarray(wx).astype(hf),
        "wh": np.ascontiguousarray(wh).astype(hf),
        "brow": np.ascontiguousarray(brow).astype(hf),
        "bbank": np.ascontiguousarray(bbank).astype(hf),
        "attn_wT": np.ascontiguousarray(attn_w.T).astype(hf),
        "attn_b": np.ascontiguousarray(attn_b[:, None], np.float32),
        "v_w": np.ascontiguousarray(v_w.T).astype(hf),
        "fc_wT": np.ascontiguousarray(fc_w.T).astype(hf),
        "fc_b": np.ascontiguousarray(fc_b[None, :]).astype(hf),
    }
    in_maps = []
    for c in range(NCORES):
        xs = x[c * BC:(c + 1) * BC, :S_run, :]
        xt = np.transpose(xs, (2, 1, 0)).reshape(IN, S_run * BC)
        xt = np.concatenate([xt, np.ones((1, S_run * BC), np.float32)], 0)
        m = dict(shared)
        m["x"] = np.ascontiguousarray(xt).astype(hf)
        in_maps.append(m)
    return in_maps


def run(inputs, S_run=S, trace=False):
    from concourse import bass_utils
    if S_run not in _CACHE:
        _CACHE[S_run] = _build(S_run)
    nc = _CACHE[S_run]
    in_maps = _prep_inputs(S_run=S_run, **inputs)
    res = bass_utils.run_bass_kernel_spmd(
        nc, in_maps, core_ids=list(range(NCORES)), trace=trace)
    out = np.concatenate([np.asarray(res.results[c]["out"], np.float32).T
                          for c in range(NCORES)], 0)
    return np.ascontiguousarray(out, np.float32), res


def kernel(**inputs):
    inputs = {k: np.asarray(v, np.float32) for k, v in inputs.items()}
    out, _ = run(inputs, S_run=S)
    return out


# revision 29
# speedup vs baseline: 4.9532x; 1.0089x over previous
"""Trainium2 Bass kernel for 10-layer LSTM + additive attention pooling + FC.

Sharding: data-parallel over batch (8 cores x 32). Per core all 10 layers run
as a wavefront (layer l computes step t = tick - l). Gates are batch-major
[4 layers x 32b, 512 gate] per PSUM bank; layers are grouped by bank
(0-3 / 4-7 / 8-9) and the three groups are software-pipelined so ACT/DVE
cell updates of one group overlap PE matmuls of the next. All matmul
operands are fp16 (1 cycle/row streaming vs 4 for fp32); PSUM accumulation
stays fp32 and the cell state c stays fp32 in SBUF. Per-layer K=1 bias
matmuls are replaced by one "expander" matmul per bank: stationary E[4,128]
with E[j, 32j:32j+32]=1 broadcasts 4 layers' bias rows across the bank's
partitions in a single 512-column pass.
"""
import sys
import numpy as np

B, S, IN, H, OUT, L = 256, 512, 27, 128, 7, 10
NCORES = 8
BC = B // NCORES  # 32
G4 = 4 * H        # 512

for _p in ("/opt/trn_rl_repo",):
    if _p not in sys.path:
        sys.path.insert(0, _p)

_CACHE = {}

GROUPS = ((0, 1, 2, 3), (4, 5, 6, 7), (8, 9))


def _build(S_run):
    from contextlib import ExitStack
    import concourse.bass as bass
    import concourse.tile as tile
    from concourse import bacc, mybir
    from concourse.masks import make_identity

    f32 = mybir.dt.float32
    fp16 = mybir.dt.float16
    NT = S_run + L - 1

    nc = bacc.Bacc("TRN2", target_bir_lowering=False, debug=False,
                   enable_asserts=False, num_devices=NCORES)

    d_x = nc.dram_tensor("x", [IN + 1, S_run * BC], fp16, kind="ExternalInput").ap()
    d_w0 = nc.dram_tensor("w0", [IN + 1, G4], fp16, kind="ExternalInput").ap()
    d_wx = nc.dram_tensor("wx", [128, 9 * G4], fp16, kind="ExternalInput").ap()
    d_wh = nc.dram_tensor("wh", [128, 10 * G4], fp16, kind="ExternalInput").ap()
    d_brow = nc.dram_tensor("brow", [1, 9 * G4], fp16, kind="ExternalInput").ap()
    d_bbank = nc.dram_tensor("bbank", [4, 3 * G4], fp16,
                             kind="ExternalInput").ap()
    d_E = nc.dram_tensor("Emat", [4, 128], fp16, kind="ExternalInput").ap()
    d_attn = nc.dram_tensor("attn_wT", [128, 128], fp16, kind="ExternalInput").ap()
    d_attnb = nc.dram_tensor("attn_b", [128, 1], f32, kind="ExternalInput").ap()
    d_vw = nc.dram_tensor("v_w", [128, 1], fp16, kind="ExternalInput").ap()
    d_fcw = nc.dram_tensor("fc_wT", [128, OUT], fp16, kind="ExternalInput").ap()
    d_fcb = nc.dram_tensor("fc_b", [1, OUT], fp16, kind="ExternalInput").ap()
    d_out = nc.dram_tensor("out", [OUT, BC], f32, kind="ExternalOutput").ap()
    import os as _os
    dbg = _os.environ.get("DEBUG_HS9") == "1"
    d_hs9 = (nc.dram_tensor("hs9", [128, S_run * BC], fp16,
                            kind="ExternalOutput").ap() if dbg else None)

    Sig = mybir.ActivationFunctionType.Sigmoid
    Tanh = mybir.ActivationFunctionType.Tanh
    Exp = mybir.ActivationFunctionType.Exp

    def active(k):
        return [l for l in range(L) if 0 <= k - l < S_run]

    with tile.TileContext(nc) as tc:
        with ExitStack() as octx:
            keep = octx.enter_context(tc.tile_pool(name="keep", bufs=1))
            hs9 = keep.tile([128, S_run * BC], fp16)
            ident = keep.tile([128, 128], fp16)
            make_identity(nc, ident[:])

            # ================= recurrent phase =================
            with ExitStack() as ctx:
                stat = ctx.enter_context(tc.tile_pool(name="stat", bufs=1))
                xT = stat.tile([IN + 1, S_run * BC], fp16)
                nc.sync.dma_start(xT[:], d_x)
                w0 = stat.tile([IN + 1, G4], fp16)
                nc.sync.dma_start(w0[:], d_w0)
                Wx = stat.tile([128, 9 * G4], fp16)
                nc.sync.dma_start(Wx[:], d_wx)
                Wh = stat.tile([128, 10 * G4], fp16)
                nc.sync.dma_start(Wh[:], d_wh)
                brow = stat.tile([1, 9 * G4], fp16)
                nc.sync.dma_start(brow[:], d_brow)
                bbank = stat.tile([4, 3 * G4], fp16)
                nc.sync.dma_start(bbank[:], d_bbank)
                E = stat.tile([4, 128], fp16)
                nc.sync.dma_start(E[:], d_E)
                ones32 = stat.tile([1, 32], fp16)
                nc.vector.memset(ones32[:], 1.0)

                psp = [ctx.enter_context(
                    tc.tile_pool(name=f"ps{g}", bufs=2, space="PSUM"))
                    for g in range(3)]
                pst = ctx.enter_context(tc.tile_pool(name="pst", bufs=2,
                                                     space="PSUM"))
                actp = [ctx.enter_context(tc.tile_pool(name=f"act{g}", bufs=2))
                        for g in range(3)]
                hbp = [ctx.enter_context(tc.tile_pool(name=f"hb{g}", bufs=2))
                       for g in range(3)]
                htp = [ctx.enter_context(tc.tile_pool(name=f"ht{g}", bufs=2))
                       for g in range(3)]
                tmpp = [ctx.enter_context(tc.tile_pool(name=f"tp{g}", bufs=2))
                        for g in range(3)]
                thp = [ctx.enter_context(tc.tile_pool(name=f"th{g}", bufs=2))
                       for g in range(3)]
                cpp = ctx.enter_context(tc.tile_pool(name="cp", bufs=1))

                c_g = [cpp.tile([128, 128], f32, name=f"c{g}")
                       for g in range(3)]
                for g in range(3):
                    nc.vector.memset(c_g[g][:], 0.0)

                # rolling state: hb tile of tick k-1, hT tile of tick k-1
                hb_prev = [None, None, None]
                hT_prev = [None, None, None]
                ps_cur = [None, None, None]

                def emit_C(g, k):
                    """transpose+copy h of tick k for group g -> hT_prev[g]."""
                    if k < 0 or hb_prev[g] is None:
                        return
                    pt = pst.tile([128, 128], fp16)
                    nc.tensor.transpose(pt[:], hb_prev[g][:], ident[:])
                    hT = htp[g].tile([128, 128], fp16)
                    nc.vector.tensor_copy(hT[:], pt[:])
                    hT_prev[g] = hT
                    if g == 2 and (0 <= k - 9 < S_run):
                        t9 = k - 9
                        dst = hs9[:].rearrange("p (b s) -> p s b", b=BC)
                        nc.gpsimd.tensor_copy(
                            dst[:, t9:t9 + 1, :], hT[:, 32:64])

                def x_operands(l, t):
                    if l == 0:
                        return xT[:, 32 * t:32 * t + 32], w0[:]
                    gp, mp = (l - 1) // 4, (l - 1) % 4
                    return (hT_prev[gp][:, 32 * mp:32 * mp + 32],
                            Wx[:, (l - 1) * G4:l * G4])

                def emit_A(g, k, full):
                    lays = [l for l in GROUPS[g] if 0 <= k - l < S_run]
                    if not lays:
                        ps_cur[g] = None
                        return
                    ps = psp[g].tile([128, G4], f32)
                    ps_cur[g] = ps
                    if full:
                        for l in lays:
                            t, m = k - l, l % 4
                            tp = (0, 32 * m)
                            o = ps[32 * m:32 * m + 32, :]
                            lx, wx_r = x_operands(l, t)
                            nc.tensor.matmul(o, lx, wx_r, start=True,
                                             stop=False, tile_position=tp)
                            if t > 0:
                                nc.tensor.matmul(
                                    o, hT_prev[g][:, 32 * m:32 * m + 32],
                                    Wh[:, l * G4:(l + 1) * G4],
                                    start=False, stop=False, tile_position=tp)
                        nc.tensor.matmul(ps[:], E[:],
                                         bbank[:, g * G4:(g + 1) * G4],
                                         start=False, stop=True,
                                         skip_group_check=True)
                    else:
                        for l in lays:
                            t, m = k - l, l % 4
                            tp = (0, 32 * m)
                            o = ps[32 * m:32 * m + 32, :]
                            lx, wx_r = x_operands(l, t)
                            nc.tensor.matmul(
                                o, lx, wx_r, start=True,
                                stop=(l == 0 and t == 0), tile_position=tp)
                            if t > 0:
                                nc.tensor.matmul(
                                    o, hT_prev[g][:, 32 * m:32 * m + 32],
                                    Wh[:, l * G4:(l + 1) * G4],
                                    start=False, stop=(l == 0),
                                    tile_position=tp)
                            if l > 0:
                                nc.tensor.matmul(
                                    o, ones32[:], brow[:, (l - 1) * G4:l * G4],
                                    start=False, stop=True, tile_position=tp)

                def cell(pr, g):
                    """gate nonlinearities + cell update on partition range pr."""
                    ps, act = ps_cur[g], act_cur[g]
                    nc.scalar.activation(act[pr, 0:384], ps[pr, 0:384], Sig)
                    nc.scalar.activation(act[pr, 384:512], ps[pr, 384:512],
                                         Tanh)
                    tt, th, cc = tt_cur[g], th_cur[g], c_g[g]
                    nc.vector.tensor_mul(tt[pr, :], act[pr, 0:128],
                                         act[pr, 384:512])
                    nc.vector.tensor_mul(cc[pr, :], act[pr, 128:256],
                                         cc[pr, :])
                    nc.vector.tensor_add(cc[pr, :], cc[pr, :], tt[pr, :])
                    nc.scalar.activation(th[pr, :], cc[pr, :], Tanh)
                    nc.vector.tensor_mul(hb_prev[g][pr, :], act[pr, 256:384],
                                         th[pr, :])

                act_cur = [None, None, None]
                tt_cur = [None, None, None]
                th_cur = [None, None, None]

                def emit_B(g, k, full):
                    lays = [l for l in GROUPS[g] if 0 <= k - l < S_run]
                    if not lays:
                        hb_prev[g] = None
                        return
                    act_cur[g] = actp[g].tile([128, G4], f32, name=f"act{g}")
                    tt_cur[g] = tmpp[g].tile([128, 128], f32, name=f"tt{g}")
                    th_cur[g] = thp[g].tile([128, 128], f32, name=f"th{g}")
                    hb_prev[g] = hbp[g].tile([128, 128], fp16, name=f"hb{g}")
                    if full:
                        cell(slice(0, 32 * len(lays)), g)
                    else:
                        m0 = min(l % 4 for l in lays)
                        m1 = max(l % 4 for l in lays)
                        # engine APs >32 partitions must start at 0/64
                        m = m0
                        while m <= m1:
                            if m % 2 == 1 or m == m1:
                                cell(slice(32 * m, 32 * (m + 1)), g)
                                m += 1
                            elif m == 0 and m1 == 3:
                                cell(slice(0, 128), g)
                                m = 4
                            elif m == 0:
                                cell(slice(0, 64), g)
                                m = 2
                            else:
                                cell(slice(64, 128), g)
                                m = 4

                for k in range(NT):
                    for g in range(3):
                        lays = [l for l in GROUPS[g] if 0 <= k - l < S_run]
                        full = len(lays) == len(GROUPS[g])
                        emit_C(g, k - 1)
                        emit_A(g, k, full)
                        emit_B(g, k, full)
                # flush: hs9 for the final tick (layer 9, t = S_run-1)
                emit_C(2, NT - 1)

            if d_hs9 is not None:
                nc.sync.dma_start(d_hs9, hs9[:])
            # ================= attention + FC =================
            with ExitStack() as ctx:
                st2 = ctx.enter_context(tc.tile_pool(name="st2", bufs=1))
                ps2 = ctx.enter_context(tc.tile_pool(name="ps2", bufs=2,
                                                     space="PSUM"))
                sc2 = ctx.enter_context(tc.tile_pool(name="sc2", bufs=2))
                aw = st2.tile([128, 128], fp16)
                nc.sync.dma_start(aw[:], d_attn)
                ab = st2.tile([128, 1], f32)
                nc.sync.dma_start(ab[:], d_attnb)
                vw = st2.tile([128, 1], fp16)
                nc.sync.dma_start(vw[:], d_vw)
                fcw = st2.tile([128, OUT], fp16)
                nc.sync.dma_start(fcw[:], d_fcw)
                fcb = st2.tile([1, OUT], fp16)
                nc.sync.dma_start(fcb[:], d_fcb)
                ones128 = st2.tile([1, 128], fp16)
                nc.vector.memset(ones128[:], 1.0)
                onesBC = st2.tile([1, BC], fp16)
                nc.vector.memset(onesBC[:], 1.0)

                # hs9 is laid out b-major: col = b * S_run + t
                wgt = st2.tile([1, S_run * BC], fp16)
                sm = st2.tile([1, BC], f32)
                for b in range(BC):
                    cs = slice(S_run * b, S_run * (b + 1))
                    pa = ps2.tile([128, S_run], f32, tag="big")
                    nc.tensor.matmul(pa[:], aw[:], hs9[:, cs],
                                     start=True, stop=True)
                    sc = sc2.tile([128, S_run], fp16, tag="sc")
                    nc.scalar.activation(sc[:], pa[:], Tanh, bias=ab[:])
                    pl = ps2.tile([1, S_run], f32, tag="pl")
                    nc.tensor.matmul(pl[:], vw[:], sc[:],
                                     start=True, stop=True)
                    nc.scalar.activation(wgt[:, cs], pl[:], Exp)
                nc.vector.tensor_reduce(
                    sm[:], wgt[:].rearrange("p (b s) -> p b s", b=BC),
                    axis=mybir.AxisListType.X, op=mybir.AluOpType.add)
                rsm = st2.tile([1, BC], f32)
                nc.vector.reciprocal(rsm[:], sm[:])
                rsm16 = st2.tile([1, BC], fp16)
                nc.vector.tensor_copy(rsm16[:], rsm[:])

                ctxv = st2.tile([128, BC], f32)
                for b in range(BC):
                    cs = slice(S_run * b, S_run * (b + 1))
                    pw = ps2.tile([128, S_run], f32, tag="big")
                    nc.tensor.matmul(pw[:], ones128[:], wgt[:, cs],
                                     start=True, stop=True)
                    wp = sc2.tile([128, S_run], f32, tag="wp")
                    nc.vector.tensor_mul(wp[:], hs9[:, cs], pw[:])
                    nc.vector.tensor_reduce(
                        ctxv[:, b:b + 1], wp[:],
                        axis=mybir.AxisListType.X, op=mybir.AluOpType.add)
                prn = ps2.tile([128, BC], f32, tag="pl")
                nc.tensor.matmul(prn[:], ones128[:], rsm16[:],
                                 start=True, stop=True)
                nc.vector.tensor_mul(ctxv[:], ctxv[:], prn[:])
                ctx16 = st2.tile([128, BC], fp16)
                nc.vector.tensor_copy(ctx16[:], ctxv[:])

                pf = ps2.tile([OUT, BC], f32, tag="pl")
                nc.tensor.matmul(pf[:], fcw[:], ctx16[:],
                                 start=True, stop=False)
                nc.tensor.matmul(pf[:], fcb[:], onesBC[:],
                                 start=False, stop=True)
                ov = sc2.tile([OUT, BC], f32, tag="ov")
                nc.vector.tensor_copy(ov[:], pf[:])
                nc.sync.dma_start(d_out, ov[:])

    nc.compile()
    return nc


def _prep_inputs(x, w_ih0, w_ih, w_hh, b_ih, b_hh, attn_w, attn_b, v_w, v_b,
                 fc_w, fc_b, S_run):
    hf = np.float16
    perm = np.concatenate([np.arange(0, H), np.arange(H, 2 * H),
                           np.arange(3 * H, 4 * H), np.arange(2 * H, 3 * H)])
    w0 = np.concatenate([w_ih0.T, (b_ih[0] + b_hh[0])[None, :]], 0)[:, perm]
    wx = np.concatenate([w_ih[l - 1].T[:, perm] for l in range(1, L)], 1)
    wh = np.concatenate([w_hh[l].T[:, perm] for l in range(L)], 1)
    brow = np.concatenate([(b_ih[l] + b_hh[l])[perm][None, :]
                           for l in range(1, L)], 1)
    bbank = np.zeros((4, 3 * G4), np.float32)
    for l in range(1, L):
        g, j = l // 4, l % 4
        bbank[j, g * G4:(g + 1) * G4] = (b_ih[l] + b_hh[l])[perm]
    Emat = np.zeros((4, 128), np.float32)
    for j in range(4):
        Emat[j, 32 * j:32 * j + 32] = 1.0
    shared = {
        "bbank": np.ascontiguousarray(bbank).astype(hf),
        "Emat": Emat.astype(hf),
        "w0": np.ascontiguousarray(w0).astype(hf),
        "wx": np.ascontiguousarray(wx).astype(hf),
        "wh": np.ascontiguousarray(wh).astype(hf),
        "brow": np.ascontiguousarray(brow).astype(hf),
        "attn_wT": np.ascontiguousarray(attn_w.T).astype(hf),
        "attn_b": np.ascontiguousarray(attn_b[:, None], np.float32),
        "v_w": np.ascontiguousarray(v_w.T).astype(hf),
        "fc_wT": np.ascontiguousarray(fc_w.T).astype(hf),
        "fc_b": np.ascontiguousarray(fc_b[None, :]).astype(hf),
    }
    in_maps = []
    for c in range(NCORES):
        xs = x[c * BC:(c + 1) * BC, :S_run, :]
        xt = np.transpose(xs, (2, 1, 0)).reshape(IN, S_run * BC)
        xt = np.concatenate([xt, np.ones((1, S_run * BC), np.float32)], 0)
        m = dict(shared)
        m["x"] = np.ascontiguousarray(xt).astype(hf)
        in_maps.append(m)
    return in_maps


def run(inputs, S_run=S, trace=False):
    from concourse import bass_utils
    if S_run not in _CACHE:
        _CACHE[S_run] = _build(S_run)
    nc = _CACHE[S_run]
    in_maps = _prep_inputs(S_run=S_run, **inputs)
    res = bass_utils.run_bass_kernel_spmd(
        nc, in_maps, core_ids=list(range(NCORES)), trace=trace)
    out = np.concatenate([np.asarray(res.results[c]["out"], np.float32).T
                          for c in range(NCORES)], 0)
    return np.ascontiguousarray(out, np.float32), res


def kernel(**inputs):
    inputs = {k: np.asarray(v, np.float32) for k, v in inputs.items()}
    out, _ = run(inputs, S_run=S)
    return out


# revision 33
# speedup vs baseline: 39.3513x; 7.9446x over previous
"""Trainium2 Bass kernel for 10-layer LSTM + additive attention pooling + FC.

Sharding: data-parallel over batch (8 cores x 32). Per core all 10 layers run
as a wavefront (layer l computes step t = tick - l). Gates are batch-major
[4 layers x 32b, 512 gate] per PSUM bank; layers are grouped by bank
(0-3 / 4-7 / 8-9) and the three groups are software-pipelined so ACT/DVE
cell updates of one group overlap PE matmuls of the next. All matmul
operands are fp16 (1 cycle/row streaming vs 4 for fp32); PSUM accumulation
stays fp32 and the cell state c stays fp32 in SBUF. Per-layer K=1 bias
matmuls are replaced by one "expander" matmul per bank: stationary E[4,128]
with E[j, 32j:32j+32]=1 broadcasts 4 layers' bias rows across the bank's
partitions in a single 512-column pass.
"""
import sys
import numpy as np

B, S, IN, H, OUT, L = 256, 512, 27, 128, 7, 10
NCORES = 8
BC = B // NCORES  # 32
G4 = 4 * H        # 512

for _p in ("/opt/trn_rl_repo",):
    if _p not in sys.path:
        sys.path.insert(0, _p)

_CACHE = {}

GROUPS = ((0, 1, 2, 3), (4, 5, 6, 7), (8, 9))


def _build(S_run):
    from contextlib import ExitStack
    import concourse.bass as bass
    import concourse.tile as tile
    from concourse import bacc, mybir
    from concourse.masks import make_identity

    f32 = mybir.dt.float32
    fp16 = mybir.dt.float16
    NT = S_run + L - 1

    nc = bacc.Bacc("TRN2", target_bir_lowering=False, debug=False,
                   enable_asserts=False, num_devices=NCORES)

    d_x = nc.dram_tensor("x", [IN + 1, S_run * BC], fp16, kind="ExternalInput").ap()
    d_w0 = nc.dram_tensor("w0", [128, G4], fp16, kind="ExternalInput").ap()
    d_wx = nc.dram_tensor("wx", [128, 9 * G4], fp16, kind="ExternalInput").ap()
    d_wh = nc.dram_tensor("wh", [128, 10 * G4], fp16, kind="ExternalInput").ap()
    d_brow = nc.dram_tensor("brow", [128, 9 * G4], fp16,
                            kind="ExternalInput").ap()
    d_E0 = nc.dram_tensor("E0", [128, 32], fp16, kind="ExternalInput").ap()
    d_attn = nc.dram_tensor("attn_wT", [128, 128], fp16, kind="ExternalInput").ap()
    d_attnb = nc.dram_tensor("attn_b", [128, 1], f32, kind="ExternalInput").ap()
    d_vw = nc.dram_tensor("v_w", [128, 1], fp16, kind="ExternalInput").ap()
    d_fcw = nc.dram_tensor("fc_wT", [128, OUT], fp16, kind="ExternalInput").ap()
    d_fcb = nc.dram_tensor("fc_b", [1, OUT], fp16, kind="ExternalInput").ap()
    d_out = nc.dram_tensor("out", [OUT, BC], f32, kind="ExternalOutput").ap()
    import os as _os
    dbg = _os.environ.get("DEBUG_HS9") == "1"
    d_hs9 = (nc.dram_tensor("hs9", [128, S_run * BC], fp16,
                            kind="ExternalOutput").ap() if dbg else None)

    Sig = mybir.ActivationFunctionType.Sigmoid
    Tanh = mybir.ActivationFunctionType.Tanh
    Exp = mybir.ActivationFunctionType.Exp

    def active(k):
        return [l for l in range(L) if 0 <= k - l < S_run]

    with tile.TileContext(nc) as tc:
        with ExitStack() as octx:
            keep = octx.enter_context(tc.tile_pool(name="keep", bufs=1))
            hs9 = keep.tile([128, S_run * BC], fp16)
            ident = keep.tile([128, 128], fp16)
            make_identity(nc, ident[:])

            # ================= recurrent phase =================
            with ExitStack() as ctx:
                stat = ctx.enter_context(tc.tile_pool(name="stat", bufs=1))
                # xT padded to K=128 so layer-0's x-matmul runs in the same
                # 128x32 array mode as all other gate matmuls (mode switches
                # drain the PE and kill col-tile concurrency). Zero the pad
                # rows; w0's pad rows are zero too so garbage contributes 0.
                xT = stat.tile([128, S_run * BC], fp16)
                nc.vector.memset(xT[0:32, :], 0.0)
                nc.vector.memset(xT[32:64, :], 0.0)
                nc.vector.memset(xT[64:128, :], 0.0)
                nc.sync.dma_start(xT[0:IN + 1, :], d_x)
                w0 = stat.tile([128, G4], fp16)
                nc.sync.dma_start(w0[:], d_w0)
                Wx = stat.tile([128, 9 * G4], fp16)
                nc.sync.dma_start(Wx[:], d_wx)
                Wh = stat.tile([128, 10 * G4], fp16)
                nc.sync.dma_start(Wh[:], d_wh)
                brow = stat.tile([128, 9 * G4], fp16)
                nc.sync.dma_start(brow[:], d_brow)
                E0 = stat.tile([128, 32], fp16)
                nc.sync.dma_start(E0[:], d_E0)

                psp = [ctx.enter_context(
                    tc.tile_pool(name=f"ps{g}", bufs=2, space="PSUM"))
                    for g in range(3)]
                pst = ctx.enter_context(tc.tile_pool(name="pst", bufs=2,
                                                     space="PSUM"))
                actp = [ctx.enter_context(tc.tile_pool(name=f"act{g}", bufs=2))
                        for g in range(3)]
                hbp = [ctx.enter_context(tc.tile_pool(name=f"hb{g}", bufs=2))
                       for g in range(3)]
                htp = [ctx.enter_context(tc.tile_pool(name=f"ht{g}", bufs=2))
                       for g in range(3)]
                tmpp = [ctx.enter_context(tc.tile_pool(name=f"tp{g}", bufs=2))
                        for g in range(3)]
                thp = [ctx.enter_context(tc.tile_pool(name=f"th{g}", bufs=2))
                       for g in range(3)]
                cpp = ctx.enter_context(tc.tile_pool(name="cp", bufs=1))

                c_g = [cpp.tile([128, 128], f32, name=f"c{g}")
                       for g in range(3)]
                for g in range(3):
                    nc.vector.memset(c_g[g][:], 0.0)

                # rolling state: hb tile of tick k-1, hT tile of tick k-1
                hb_prev = [None, None, None]
                hT_prev = [None, None, None]
                ps_cur = [None, None, None]

                def emit_C(g, k):
                    """transpose+copy h of tick k for group g -> hT_prev[g]."""
                    if k < 0 or hb_prev[g] is None:
                        return
                    pt = pst.tile([128, 128], fp16)
                    nc.tensor.transpose(pt[:], hb_prev[g][:], ident[:])
                    hT = htp[g].tile([128, 128], fp16)
                    nc.vector.tensor_copy(hT[:], pt[:])
                    hT_prev[g] = hT
                    if g == 2 and (0 <= k - 9 < S_run):
                        t9 = k - 9
                        dst = hs9[:].rearrange("p (b s) -> p s b", b=BC)
                        nc.gpsimd.tensor_copy(
                            dst[:, t9:t9 + 1, :], hT[:, 32:64])

                def x_operands(l, t):
                    if l == 0:
                        return xT[:, 32 * t:32 * t + 32], w0[:]
                    gp, mp = (l - 1) // 4, (l - 1) % 4
                    return (hT_prev[gp][:, 32 * mp:32 * mp + 32],
                            Wx[:, (l - 1) * G4:l * G4])

                def emit_A(g, k, full):
                    # All matmuls run in uniform 128x32 array mode, phase
                    # ordered (x / h / bias) so consecutive matmuls target
                    # distinct 32-col tiles and execute concurrently.
                    lays = [l for l in GROUPS[g] if 0 <= k - l < S_run]
                    if not lays:
                        ps_cur[g] = None
                        return
                    ps = psp[g].tile([128, G4], f32)
                    ps_cur[g] = ps
                    for l in lays:
                        t, m = k - l, l % 4
                        o = ps[32 * m:32 * m + 32, :]
                        lx, wx_r = x_operands(l, t)
                        nc.tensor.matmul(o, lx, wx_r, start=True,
                                         stop=(l == 0 and t == 0),
                                         tile_position=(0, 32 * m))
                    for l in lays:
                        t, m = k - l, l % 4
                        if t > 0:
                            nc.tensor.matmul(
                                ps[32 * m:32 * m + 32, :],
                                hT_prev[g][:, 32 * m:32 * m + 32],
                                Wh[:, l * G4:(l + 1) * G4],
                                start=False, stop=(l == 0),
                                tile_position=(0, 32 * m))
                    for l in lays:
                        m = l % 4
                        if l > 0:
                            nc.tensor.matmul(
                                ps[32 * m:32 * m + 32, :], E0[:],
                                brow[:, (l - 1) * G4:l * G4],
                                start=False, stop=True,
                                tile_position=(0, 32 * m))

                def cell(pr, g):
                    """gate nonlinearities + cell update on partition range pr."""
                    ps, act = ps_cur[g], act_cur[g]
                    nc.scalar.activation(act[pr, 0:384], ps[pr, 0:384], Sig)
                    nc.scalar.activation(act[pr, 384:512], ps[pr, 384:512],
                                         Tanh)
                    tt, th, cc = tt_cur[g], th_cur[g], c_g[g]
                    nc.vector.tensor_mul(tt[pr, :], act[pr, 0:128],
                                         act[pr, 384:512])
                    nc.vector.tensor_mul(cc[pr, :], act[pr, 128:256],
                                         cc[pr, :])
                    nc.vector.tensor_add(cc[pr, :], cc[pr, :], tt[pr, :])
                    nc.scalar.activation(th[pr, :], cc[pr, :], Tanh)
                    nc.vector.tensor_mul(hb_prev[g][pr, :], act[pr, 256:384],
                                         th[pr, :])

                act_cur = [None, None, None]
                tt_cur = [None, None, None]
                th_cur = [None, None, None]

                def emit_B(g, k, full):
                    lays = [l for l in GROUPS[g] if 0 <= k - l < S_run]
                    if not lays:
                        hb_prev[g] = None
                        return
                    act_cur[g] = actp[g].tile([128, G4], f32, name=f"act{g}")
                    tt_cur[g] = tmpp[g].tile([128, 128], f32, name=f"tt{g}")
                    th_cur[g] = thp[g].tile([128, 128], f32, name=f"th{g}")
                    hb_prev[g] = hbp[g].tile([128, 128], fp16, name=f"hb{g}")
                    if full:
                        cell(slice(0, 32 * len(lays)), g)
                    else:
                        m0 = min(l % 4 for l in lays)
                        m1 = max(l % 4 for l in lays)
                        # engine APs >32 partitions must start at 0/64
                        m = m0
                        while m <= m1:
                            if m % 2 == 1 or m == m1:
                                cell(slice(32 * m, 32 * (m + 1)), g)
                                m += 1
                            elif m == 0 and m1 == 3:
                                cell(slice(0, 128), g)
                                m = 4
                            elif m == 0:
                                cell(slice(0, 64), g)
                                m = 2
                            else:
                                cell(slice(64, 128), g)
                                m = 4

                for k in range(NT):
                    for g in range(3):
                        lays = [l for l in GROUPS[g] if 0 <= k - l < S_run]
                        full = len(lays) == len(GROUPS[g])
                        emit_C(g, k - 1)
                        emit_A(g, k, full)
                        emit_B(g, k, full)
                # flush: hs9 for the final tick (layer 9, t = S_run-1)
                emit_C(2, NT - 1)

            if d_hs9 is not None:
                nc.sync.dma_start(d_hs9, hs9[:])
            # ================= attention + FC =================
            with ExitStack() as ctx:
                st2 = ctx.enter_context(tc.tile_pool(name="st2", bufs=1))
                ps2 = ctx.enter_context(tc.tile_pool(name="ps2", bufs=2,
                                                     space="PSUM"))
                sc2 = ctx.enter_context(tc.tile_pool(name="sc2", bufs=2))
                aw = st2.tile([128, 128], fp16)
                nc.sync.dma_start(aw[:], d_attn)
                ab = st2.tile([128, 1], f32)
                nc.sync.dma_start(ab[:], d_attnb)
                vw = st2.tile([128, 1], fp16)
                nc.sync.dma_start(vw[:], d_vw)
                fcw = st2.tile([128, OUT], fp16)
                nc.sync.dma_start(fcw[:], d_fcw)
                fcb = st2.tile([1, OUT], fp16)
                nc.sync.dma_start(fcb[:], d_fcb)
                ones128 = st2.tile([1, 128], fp16)
                nc.vector.memset(ones128[:], 1.0)
                onesBC = st2.tile([1, BC], fp16)
                nc.vector.memset(onesBC[:], 1.0)

                # hs9 is laid out b-major: col = b * S_run + t
                wgt = st2.tile([1, S_run * BC], fp16)
                sm = st2.tile([1, BC], f32)
                for b in range(BC):
                    cs = slice(S_run * b, S_run * (b + 1))
                    pa = ps2.tile([128, S_run], f32, tag="big")
                    nc.tensor.matmul(pa[:], aw[:], hs9[:, cs],
                                     start=True, stop=True)
                    sc = sc2.tile([128, S_run], fp16, tag="sc")
                    nc.scalar.activation(sc[:], pa[:], Tanh, bias=ab[:])
                    pl = ps2.tile([1, S_run], f32, tag="pl")
                    nc.tensor.matmul(pl[:], vw[:], sc[:],
                                     start=True, stop=True)
                    nc.scalar.activation(wgt[:, cs], pl[:], Exp)
                nc.vector.tensor_reduce(
                    sm[:], wgt[:].rearrange("p (b s) -> p b s", b=BC),
                    axis=mybir.AxisListType.X, op=mybir.AluOpType.add)
                rsm = st2.tile([1, BC], f32)
                nc.vector.reciprocal(rsm[:], sm[:])
                rsm16 = st2.tile([1, BC], fp16)
                nc.vector.tensor_copy(rsm16[:], rsm[:])

                ctxv = st2.tile([128, BC], f32)
                for b in range(BC):
                    cs = slice(S_run * b, S_run * (b + 1))
                    pw = ps2.tile([128, S_run], f32, tag="big")
                    nc.tensor.matmul(pw[:], ones128[:], wgt[:, cs],
                                     start=True, stop=True)
                    wp = sc2.tile([128, S_run], f32, tag="wp")
                    nc.vector.tensor_mul(wp[:], hs9[:, cs], pw[:])
                    nc.vector.tensor_reduce(
                        ctxv[:, b:b + 1], wp[:],
                        axis=mybir.AxisListType.X, op=mybir.AluOpType.add)
                prn = ps2.tile([128, BC], f32, tag="pl")
                nc.tensor.matmul(prn[:], ones128[:], rsm16[:],
                                 start=True, stop=True)
                nc.vector.tensor_mul(ctxv[:], ctxv[:], prn[:])
                ctx16 = st2.tile([128, BC], fp16)
                nc.vector.tensor_copy(ctx16[:], ctxv[:])

                pf = ps2.tile([OUT, BC], f32, tag="pl")
                nc.tensor.matmul(pf[:], fcw[:], ctx16[:],
                                 start=True, stop=False)
                nc.tensor.matmul(pf[:], fcb[:], onesBC[:],
                                 start=False, stop=True)
                ov = sc2.tile([OUT, BC], f32, tag="ov")
                nc.vector.tensor_copy(ov[:], pf[:])
                nc.sync.dma_start(d_out, ov[:])

    nc.compile()
    return nc


def _prep_inputs(x, w_ih0, w_ih, w_hh, b_ih, b_hh, attn_w, attn_b, v_w, v_b,
                 fc_w, fc_b, S_run):
    hf = np.float16
    perm = np.concatenate([np.arange(0, H), np.arange(H, 2 * H),
                           np.arange(3 * H, 4 * H), np.arange(2 * H, 3 * H)])
    w0 = np.zeros((128, G4), np.float32)
    w0[:IN + 1] = np.concatenate(
        [w_ih0.T, (b_ih[0] + b_hh[0])[None, :]], 0)[:, perm]
    wx = np.concatenate([w_ih[l - 1].T[:, perm] for l in range(1, L)], 1)
    wh = np.concatenate([w_hh[l].T[:, perm] for l in range(L)], 1)
    brow = np.zeros((128, 9 * G4), np.float32)
    brow[0] = np.concatenate([(b_ih[l] + b_hh[l])[perm]
                              for l in range(1, L)], 0)
    E0mat = np.zeros((128, 32), np.float32)
    E0mat[0, :] = 1.0
    shared = {
        "E0": E0mat.astype(hf),
        "w0": np.ascontiguousarray(w0).astype(hf),
        "wx": np.ascontiguousarray(wx).astype(hf),
        "wh": np.ascontiguousarray(wh).astype(hf),
        "brow": np.ascontiguousarray(brow).astype(hf),
        "attn_wT": np.ascontiguousarray(attn_w.T).astype(hf),
        "attn_b": np.ascontiguousarray(attn_b[:, None], np.float32),
        "v_w": np.ascontiguousarray(v_w.T).astype(hf),
        "fc_wT": np.ascontiguousarray(fc_w.T).astype(hf),
        "fc_b": np.ascontiguousarray(fc_b[None, :]).astype(hf),
    }
    in_maps = []
    for c in range(NCORES):
        xs = x[c * BC:(c + 1) * BC, :S_run, :]
        xt = np.transpose(xs, (2, 1, 0)).reshape(IN, S_run * BC)
        xt = np.concatenate([xt, np.ones((1, S_run * BC), np.float32)], 0)
        m = dict(shared)
        m["x"] = np.ascontiguousarray(xt).astype(hf)
        in_maps.append(m)
    return in_maps


def run(inputs, S_run=S, trace=False):
    from concourse import bass_utils
    if S_run not in _CACHE:
        _CACHE[S_run] = _build(S_run)
    nc = _CACHE[S_run]
    in_maps = _prep_inputs(S_run=S_run, **inputs)
    res = bass_utils.run_bass_kernel_spmd(
        nc, in_maps, core_ids=list(range(NCORES)), trace=trace)
    out = np.concatenate([np.asarray(res.results[c]["out"], np.float32).T
                          for c in range(NCORES)], 0)
    return np.ascontiguousarray(out, np.float32), res


def kernel(**inputs):
    inputs = {k: np.asarray(v, np.float32) for k, v in inputs.items()}
    out, _ = run(inputs, S_run=S)
    return out
